# revision 1
# baseline (speedup 1.0000x reference)
"""DigitCaps dynamic-routing kernel for 8 TRN2 NeuronCores.

Strategy: shard the C=1152 input capsules across the 8 cores (144 each) and
keep the full batch B=256 on every core.  The routing iterations use the
factored form (never materializing u_hat = x @ W, which would be 189 MB):

  s[b,u,o]    = sum_{c,i} x[b,i,c] * (coef[c,u] * W[c,u,o,i])     (matmul, K=(c,i))
  v           = squash(s)
  G[ci,uo]    = sum_b x[b,i,c] * v[b,u,o]                          (matmul, K=b)
  agr[c,u]    = (1/B) * sum_{o,i} W[c,u,o,i] * G[(c,i),(u,o)]      (mult + selector matmul)
  b_logits   += agr ; coef = softmax_u(b_logits)                   (tiny, c-local)

Only cross-core traffic: AllGather of the per-core partial s ([256,160] f32)
once per routing iteration (4 total).  The c-sharded agreement/logits state is
fully core-local.  Iteration 1's uniform coef=0.1 is folded into a 0.1
pre-scale of the x operand used by the s-matmul (and cancelled for later
iterations by scaling the coefficient-expansion constant by 10).

Precision: all s/G matmuls run as float32r (~2 cyc/row measured vs 4+ for
fp32; free dims padded to 256), including the final iteration's — measured
output error stays ~3e-4.  The squash, logits, softmax and coefficient
expansion stay fp32; the collective wire is fp16.
"""

import os
import sys

# Prefer the Mesh collective algorithm for the small (80KB) AllGathers: RDH
# measured ~12us vs Mesh ~8us at this size.  Harmless if the runtime ignores it.
os.environ.setdefault("NEURON_RT_DBG_RDH_CC", "0")

if "/opt/trn_rl_repo" not in sys.path:
    sys.path.insert(0, "/opt/trn_rl_repo")

import numpy as np

import concourse.bacc as bacc
import concourse.tile as tile
from concourse import mybir
from concourse.bass_utils import run_bass_kernel_spmd

F32 = mybir.dt.float32
F32R = mybir.dt.float32r
F16 = mybir.dt.float16
WIRE_DT = F16  # collective wire dtype (algorithm is RDH for either dtype; fp16 halves the gather)

B = 256          # batch
IU = 8           # in_unit (i)
C = 1152         # input capsules
U = 10           # output capsules
O = 16           # unit size
N_CORES = 8
CL = C // N_CORES          # 144 local capsules
CI = CL * IU               # 1152 local (c,i) rows
K = CI // 128              # 9 contraction tiles
UO = U * O                 # 160
UOP = 256                  # padded free dim so float32r runs at 1 cyc/row
NROUTE = 4
N_WARM = 24   # PE keep-warm dummy matmuls per routing iteration

# matmul dtype for the coefficient-path matmuls (s iters 0-2, G)
FAST_LAST_S = True  # fp32r on the final s-matmul too (~4us tail saving, ~+2e-4 err)


def _mm(nc, out, lhsT, rhs, start, stop, fast):
    if fast:
        lhsT = lhsT.bitcast(F32R)
        rhs = rhs.bitcast(F32R)
    nc.tensor.matmul(out, lhsT=lhsT, rhs=rhs, start=start, stop=stop)


def _build_program():
    nc = bacc.Bacc(
        "TRN2",
        target_bir_lowering=False,
        debug=False,
        enable_asserts=False,
        num_devices=N_CORES,
    )

    xp_d = nc.dram_tensor("xp", [128, K * B], F32, kind="ExternalInput").ap()
    xb_d = nc.dram_tensor("xb", [128, 2 * CI], F32, kind="ExternalInput").ap()
    w1_d = nc.dram_tensor("w1", [128, K * UOP], F32, kind="ExternalInput").ap()
    zc_d = nc.dram_tensor("zc", [128, K * (UOP - UO)], F32, kind="ExternalInput").ap()
    sel_d = nc.dram_tensor("sel", [128, 16], F32, kind="ExternalInput").ap()
    exp_d = nc.dram_tensor("exp16", [16, 128], F32, kind="ExternalInput").ap()
    out_d = nc.dram_tensor("out", [B, U, O, 1], F32, kind="ExternalOutput").ap()

    with tile.TileContext(nc) as tc:
        with (
            tc.tile_pool(name="persist", bufs=1) as pp,
            tc.tile_pool(name="work", bufs=2) as wp,
            tc.tile_pool(name="sps", bufs=1, space="PSUM") as sps,
            tc.tile_pool(name="gps", bufs=1, space="PSUM") as gps,
            tc.tile_pool(name="aps", bufs=1, space="PSUM") as aps,
            tc.tile_pool(name="cxps", bufs=1, space="PSUM") as cxps,
            tc.tile_pool(name="dram", bufs=2, space="DRAM") as dram,
        ):
            # ---- persistent tiles; padded regions zeroed once ----
            xp_s = pp.tile([128, K * B], F32, tag="xp")
            xb_s = pp.tile([128, 2 * CI], F32, tag="xb")
            w1_s = pp.tile([128, K * UOP], F32, tag="w1")     # [.., k*256+uo], pad 160:256
            weff_s = pp.tile([128, K * UOP], F32, tag="weff")
            v_s = pp.tile([128, 2 * UOP], F32, tag="v")       # [.., t*256+uo]
            v_last = pp.tile([128, 2 * UO], F32, tag="vlast")
            weff_last = pp.tile([128, K * UO], F32, tag="wefflast")
            pm2_s = pp.tile([128, K * U], F32, tag="pm2")     # o-reduced G*W
            sel_s = pp.tile([128, 16], F32, tag="sel")
            exp_s = pp.tile([16, 128], F32, tag="exp16")
            b_state = pp.tile([16, K * U], F32, tag="bstate")
            scr = pp.tile([128, 2], F32, tag="scr")           # ACT table prewarm scratch

            nc.gpsimd.memset(scr[:], 1.0)


            for j in range(3):
                kb3, ku3 = 3 * B, 3 * UOP
                nc.sync.dma_start(
                    xp_s[:, j * kb3:(j + 1) * kb3].bitcast(F32R),
                    xp_d[:, j * kb3:(j + 1) * kb3].bitcast(F32R),
                )
                nc.scalar.dma_start(
                    w1_s[:, j * ku3:(j + 1) * ku3].bitcast(F32R),
                    w1_d[:, j * ku3:(j + 1) * ku3].bitcast(F32R),
                )
            nc.scalar.dma_start(
                weff_s[:].rearrange("p (k q) -> p k q", k=K)[:, :, UO:].bitcast(F32R),
                zc_d.rearrange("p (k q) -> p k q", k=K).bitcast(F32R),
            )
            nc.scalar.dma_start(
                v_s[:].rearrange("p (t q) -> p t q", t=2)[:, :, UO:].bitcast(F32R),
                zc_d[:, : 2 * (UOP - UO)].rearrange("p (t q) -> p t q", t=2).bitcast(F32R),
            )
            nc.sync.dma_start(xb_s[:, :CI].bitcast(F32R), xb_d[:, :CI].bitcast(F32R))
            nc.sync.dma_start(xb_s[:, CI:].bitcast(F32R), xb_d[:, CI:].bitcast(F32R))
            nc.scalar.dma_start(sel_s[:], sel_d)
            nc.scalar.dma_start(exp_s[:], exp_d)

            weff = w1_s  # iteration 0 uses raw W (coef folded into xp scale)

            for r in range(NROUTE):
                last = r == NROUTE - 1
                fast_s = not last or FAST_LAST_S
                # ---- s partial: [b,(u,o)] += xp^T @ weff over (c,i) tiles ----
                wdt = WIRE_DT
                s_stage = wp.tile([128, 2 * UO], wdt, tag="s_stage" + ("_l" if last else ""))
                wstride = UOP if fast_s else UO
                s_ps = sps.tile([128, 2 * UOP], F32, tag="s_ps")
                for g in range(2):
                    for k in range(K):
                        out_ap = s_ps[:, g * UOP:(g + 1) * UOP]
                        _mm(
                            nc, out_ap if fast_s else out_ap[:, :UO],
                            xp_s[:, k * B + g * 128: k * B + (g + 1) * 128],
                            weff[:, k * wstride: k * wstride + wstride],
                            start=(k == 0), stop=(k == K - 1), fast=fast_s,
                        )
                    nc.vector.tensor_copy(
                        s_stage[:, g * UO:(g + 1) * UO],
                        s_ps[:, g * UOP: g * UOP + UO],
                    )

                # ---- AllGather partials, local 8-way tree sum ----
                cc_in = dram.tile([B, UO], wdt, tag="cc_in" + ("_l" if last else ""))
                cc_out = dram.tile(
                    [N_CORES * B, UO], wdt, tag="cc_out" + ("_l" if last else ""),
                    addr_space="Shared",
                )
                nc.sync.dma_start(
                    cc_in.opt().rearrange("(g p) f -> p g f", g=2), s_stage[:]
                )
                nc.gpsimd.collective_compute(
                    "AllGather",
                    mybir.AluOpType.bypass,
                    replica_groups=[list(range(N_CORES))],
                    ins=[cc_in.opt()],
                    outs=[cc_out.opt()],
                )
                sg_s = wp.tile([128, N_CORES * 2 * UO], wdt, tag="sg" + ("_l" if last else ""))
                cc_view = cc_out.opt().rearrange(
                    "(q g p) f -> q p g f", q=8, g=2
                )  # q: rank, g: b-half
                qf = 2 * UO
                engs = [nc.sync, nc.scalar, nc.gpsimd, nc.sync,
                        nc.scalar, nc.gpsimd, nc.sync, nc.scalar]
                for q in range(8):
                    engs[q].dma_start(sg_s[:, q * qf:(q + 1) * qf], cc_view[q])
                t1 = wp.tile([128, 4 * 2 * UO], F32, tag="t1")
                nc.vector.tensor_add(t1[:], sg_s[:, :4 * 2 * UO], sg_s[:, 4 * 2 * UO:])
                t2 = wp.tile([128, 2 * 2 * UO], F32, tag="t2")
                nc.vector.tensor_add(t2[:], t1[:, :2 * 2 * UO], t1[:, 2 * 2 * UO:])
                s_sb = wp.tile([128, 2 * UO], F32, tag="s_sb")
                nc.vector.tensor_add(s_sb[:], t2[:, :2 * UO], t2[:, 2 * UO:])

                # ---- squash: v = s * sqrt(n2) / (1 + n2) ----
                sq = wp.tile([128, 2 * UO], F32, tag="sq")
                nc.vector.tensor_mul(sq[:], s_sb[:], s_sb[:])
                n2 = wp.tile([128, 2 * U], F32, tag="n2")
                nc.vector.reduce_sum(
                    n2[:], sq[:].rearrange("p (t u o) -> p (t u) o", t=2, u=U),
                    axis=mybir.AxisListType.X,
                )
                rt = wp.tile([128, 2 * U], F32, tag="rt")
                nc.scalar.sqrt(rt[:], n2[:])
                if not last:
                    # prewarm the Exp ACT table while G/sel run (dep on rt orders it)
                    nc.scalar.activation(
                        scr[:, 1:2], rt[:, 0:1], mybir.ActivationFunctionType.Exp
                    )
                dn = wp.tile([128, 2 * U], F32, tag="dn")
                nc.vector.tensor_scalar_add(dn[:], n2[:], 1.0)
                rd = wp.tile([128, 2 * U], F32, tag="rd")
                nc.vector.reciprocal(rd[:], dn[:])
                f = wp.tile([128, 2 * U], F32, tag="f")
                nc.vector.tensor_mul(f[:], rt[:], rd[:])
                if last:
                    v_out = v_last[:].rearrange("p (t u o) -> p t u o", t=2, u=U)
                else:
                    v_out = v_s[:].rearrange("p (t q o) -> p t q o", t=2, q=16)[:, :, :U, :].bitcast(F32R)
                nc.vector.tensor_mul(
                    v_out,
                    s_sb[:].rearrange("p (t u o) -> p t u o", t=2, u=U),
                    f[:].rearrange("p (t u) -> p t u", t=2).unsqueeze(3).broadcast_to((128, 2, U, O)),
                )

                if last:
                    nc.sync.dma_start(
                        out_d.rearrange("(g p) u o one -> p g (u o one)", g=2),
                        v_last[:].rearrange("p (t f) -> p t f", t=2),
                    )
                    break

                # ---- G[(c,i),(u,o)] = sum_b x*v ; pm2 = sum_o G*W ; agr = sel^T pm2 ----
                g_ps = gps.tile([128, K * UOP], F32, tag="g_ps")
                for m in range(K):
                    for t in range(2):
                        _mm(
                            nc, g_ps[:, m * UOP:(m + 1) * UOP],
                            xb_s[:, t * CI + m * 128: t * CI + (m + 1) * 128],
                            v_s[:, t * UOP:(t + 1) * UOP],
                            start=(t == 0), stop=(t == 1), fast=True,
                        )
                pm = wp.tile([128, K * UO], F32, tag="pm")
                for j in range(3):
                    nc.vector.tensor_mul(
                        pm[:].rearrange("p (k f) -> p k f", k=K)[:, 3 * j:3 * j + 3, :],
                        g_ps[:].rearrange("p (k q) -> p k q", k=K)[:, 3 * j:3 * j + 3, :UO],
                        w1_s[:].rearrange("p (k q) -> p k q", k=K)[:, 3 * j:3 * j + 3, :UO],
                    )
                    nc.vector.reduce_sum(
                        pm2_s[:].rearrange("p (k u) -> p k u", k=K)[:, 3 * j:3 * j + 3, :],
                        pm[:].rearrange("p (k u o) -> p k u o", k=K, u=U)[:, 3 * j:3 * j + 3, :, :],
                        axis=mybir.AxisListType.X,
                    )
                a_ps = aps.tile([16, K * U], F32, tag="a_ps")
                nc.tensor.matmul(a_ps[:], lhsT=sel_s[:], rhs=pm2_s[:], start=True, stop=True)

                # ---- logits update + softmax over u (c-local, tiny) ----
                if r == 0:
                    nc.vector.tensor_copy(b_state[:], a_ps[:])
                else:
                    nc.vector.tensor_add(b_state[:], b_state[:], a_ps[:])
                eb = wp.tile([16, K * U], F32, tag="eb")
                nc.scalar.activation(eb[:], b_state[:], mybir.ActivationFunctionType.Exp)
                # prewarm the Sqrt ACT table for the next squash
                nc.scalar.activation(
                    scr[:16, 0:1], eb[:, 0:1], mybir.ActivationFunctionType.Sqrt
                )
                den = wp.tile([16, K], F32, tag="den")
                nc.vector.reduce_sum(
                    den[:], eb[:].rearrange("p (k u) -> p k u", k=K),
                    axis=mybir.AxisListType.X,
                )
                rden = wp.tile([16, K], F32, tag="rden")
                nc.vector.reciprocal(rden[:], den[:])
                cnorm = wp.tile([16, K * U], F32, tag="cnorm")
                nc.vector.tensor_mul(
                    cnorm[:].rearrange("p (k u) -> p k u", k=K),
                    eb[:].rearrange("p (k u) -> p k u", k=K),
                    rden[:].unsqueeze(2).broadcast_to((16, K, U)),
                )

                # ---- expand coef to (c,i) partitions; W_eff = W * coef ----
                cx_ps = cxps.tile([128, K * U], F32, tag="cx")
                nc.tensor.matmul(cx_ps[:], lhsT=exp_s[:], rhs=cnorm[:], start=True, stop=True)
                cx_sb = wp.tile([128, K * U], F32, tag="cx_sb")
                nc.vector.tensor_copy(cx_sb[:], cx_ps[:])
                if r < NROUTE - 2 or FAST_LAST_S:
                    weff_out = weff_s[:].rearrange("p (k q o) -> p k q o", k=K, q=16)[:, :, :U, :].bitcast(F32R)
                    weff = weff_s
                else:
                    weff_out = weff_last[:].rearrange("p (k u o) -> p k u o", k=K, u=U)
                    weff = weff_last
                nc.vector.tensor_mul(
                    weff_out,
                    w1_s[:].rearrange("p (k q o) -> p k q o", k=K, q=16)[:, :, :U, :],
                    cx_ps[:].rearrange("p (k u) -> p k u", k=K).unsqueeze(3).broadcast_to((128, K, U, O)),
                )

    nc.compile()
    return nc


_PROGRAM_CACHE = {}


def _get_program():
    if "nc" not in _PROGRAM_CACHE:
        _PROGRAM_CACHE["nc"] = _build_program()
    return _PROGRAM_CACHE["nc"]


def _make_in_maps(x, W):
    x = np.ascontiguousarray(x, dtype=np.float32)
    W = np.ascontiguousarray(W, dtype=np.float32)
    sel = np.zeros((128, 16), dtype=np.float32)
    for p in range(128):
        sel[p, p // IU] = 1.0 / B
    exp16 = np.zeros((16, 128), dtype=np.float32)
    for p in range(128):
        exp16[p // IU, p] = 10.0  # cancels the 0.1 pre-scale of xp

    in_maps = []
    for core in range(N_CORES):
        c0 = core * CL
        xc = x[:, :, c0:c0 + CL]                    # [B, I, CL]
        Wc = W[c0:c0 + CL]                          # [CL, U, O, I]
        # xp[p, k*B + b] = 0.1 * x[b, i, c], ci = k*128+p = c_rel*8+i
        xp = 0.1 * xc.transpose(2, 1, 0).reshape(CI, B)
        xp = np.ascontiguousarray(
            xp.reshape(K, 128, B).transpose(1, 0, 2).reshape(128, K * B)
        )
        # xb[p, t*CI + ci] = x[t*128+p, i, c]
        xb = xc.transpose(0, 2, 1).reshape(B, CI)
        xb = np.ascontiguousarray(
            xb.reshape(2, 128, CI).transpose(1, 0, 2).reshape(128, 2 * CI)
        )
        # w1[p, k*UOP + uo] = W[c, u, o, i], zero-padded to UOP per k-tile
        w1 = Wc.transpose(0, 3, 1, 2).reshape(CI, UO).reshape(K, 128, UO)
        w1p = np.zeros((128, K, UOP), dtype=np.float32)
        w1p[:, :, :UO] = w1.transpose(1, 0, 2)
        w1p = np.ascontiguousarray(w1p.reshape(128, K * UOP))
        zc = np.zeros((128, K * (UOP - UO)), dtype=np.float32)
        in_maps.append(
            {"xp": xp, "xb": xb, "w1": w1p, "sel": sel, "exp16": exp16, "zc": zc}
        )
    return in_maps


def kernel(x, W, _trace=False, _trace_kwargs=None):
    nc = _get_program()
    in_maps = _make_in_maps(x, W)
    res = run_bass_kernel_spmd(
        nc, in_maps, core_ids=list(range(N_CORES)), trace=_trace,
        **(_trace_kwargs or {}),
    )
    out = res.results[0]["out"].astype(np.float32).reshape(B, U, O, 1)
    if _trace:
        kernel.last_results = res
    return out



# revision 9
# speedup vs baseline: 1.2006x; 1.2006x over previous
"""DigitCaps dynamic-routing kernel for 8 TRN2 NeuronCores.

Strategy: shard the C=1152 input capsules across the 8 cores (144 each) and
keep the full batch B=256 on every core.  The routing iterations use the
factored form (never materializing u_hat = x @ W, which would be 189 MB):

  s[b,u,o]    = sum_{c,i} x[b,i,c] * (coef[c,u] * W[c,u,o,i])     (matmul, K=(c,i))
  v           = squash(s)
  G[ci,uo]    = sum_b x[b,i,c] * v[b,u,o]                          (matmul, K=b)
  agr[ci',u]  = (1/B) * sum_{o} W*G summed over i via sel2 matmul  (replicated
                to all 128 (c,i) partitions so softmax runs there directly)
  b_logits   += agr ; coef = softmax_u(b_logits)                   (c-local)

Cross-core traffic per routing iteration: one fp16 AllReduce of the partial
s ([256,160], 80KB) — the collective does the 8-way sum, so no gather DMAs or
local tree-sum.  The final iteration uses a ReduceScatter instead: each core
squashes only its 32-row batch shard and returns a sharded output that the
host concatenates.

All matmul operands are bf16 (x, W, W_eff, v); accumulation stays fp32 in
PSUM.  Measured output error ~2.8e-3 vs the fp32 reference (tolerance 2e-2).
Iteration 0's uniform coef=0.1 is applied as a 0.1 scale in the PSUM->wire
copy of s.  Elementwise work is split across the vector and gpsimd engines;
the G accumulators use 3 PSUM tiles so the agreement multiply overlaps the
tail of the G matmuls.
"""

import os
import sys

# Prefer the Mesh collective algorithm (RDH measured slower at this size).
os.environ.setdefault("NEURON_RT_DBG_RDH_CC", "0")

if "/opt/trn_rl_repo" not in sys.path:
    sys.path.insert(0, "/opt/trn_rl_repo")

import numpy as np
import ml_dtypes

import concourse.bacc as bacc
import concourse.tile as tile
from concourse import mybir
from concourse.bass_utils import run_bass_kernel_spmd

F32 = mybir.dt.float32
F16 = mybir.dt.float16
BF16 = mybir.dt.bfloat16

B = 256          # batch
IU = 8           # in_unit (i)
C = 1152         # input capsules
U = 10           # output capsules
O = 16           # unit size
N_CORES = 8
CL = C // N_CORES          # 144 local capsules
CI = CL * IU               # 1152 local (c,i) rows
K = CI // 128              # 9 contraction tiles
UO = U * O                 # 160
BL = B // N_CORES          # 32 batch rows per core in the scattered output
NROUTE = 4


def _build_program():
    nc = bacc.Bacc(
        "TRN2",
        target_bir_lowering=False,
        debug=False,
        enable_asserts=False,
        num_devices=N_CORES,
    )

    xp_d = nc.dram_tensor("xp", [128, K * B], BF16, kind="ExternalInput").ap()
    xb_d = nc.dram_tensor("xb", [128, 2 * CI], BF16, kind="ExternalInput").ap()
    w1_d = nc.dram_tensor("w1", [128, K * UO], BF16, kind="ExternalInput").ap()
    sel2_d = nc.dram_tensor("sel2", [128, 128], F32, kind="ExternalInput").ap()
    out_d = nc.dram_tensor("out", [BL, U, O, 1], F32, kind="ExternalOutput").ap()

    with tile.TileContext(nc) as tc:
        with (
            tc.tile_pool(name="persist", bufs=1) as pp,
            tc.tile_pool(name="work", bufs=2) as wp,
            tc.tile_pool(name="sps", bufs=1, space="PSUM") as sps,
            tc.tile_pool(name="gps", bufs=1, space="PSUM") as gps,
            tc.tile_pool(name="aps", bufs=1, space="PSUM") as aps,
            tc.tile_pool(name="dram", bufs=2, space="DRAM") as dram,
        ):
            # ---- persistent tiles ----
            xp_s = pp.tile([128, K * B], BF16, tag="xp")
            xb_s = pp.tile([128, 2 * CI], BF16, tag="xb")
            w1_s = pp.tile([128, K * UO], BF16, tag="w1")
            weff_s = pp.tile([128, K * UO], BF16, tag="weff")
            v_s = pp.tile([128, 2 * UO], BF16, tag="v")
            sel2_s = pp.tile([128, 128], F32, tag="sel2")
            pm2_s = pp.tile([128, K * U], F32, tag="pm2")
            b_state = pp.tile([128, K * U], F32, tag="bstate")
            scr = pp.tile([128, 2], F32, tag="scr")   # ACT table prewarm scratch

            nc.gpsimd.memset(scr[:], 1.0)

            # ---- input loads: xp/w1 first (iter-0 s-matmul), xb/sel2 later ----
            kb3, ku3 = 3 * B, 3 * UO
            for j in range(3):
                nc.sync.dma_start(
                    xp_s[:, j * kb3:(j + 1) * kb3], xp_d[:, j * kb3:(j + 1) * kb3]
                )
                nc.scalar.dma_start(
                    w1_s[:, j * ku3:(j + 1) * ku3], w1_d[:, j * ku3:(j + 1) * ku3]
                )
            nc.gpsimd.dma_start(xb_s[:, :CI], xb_d[:, :CI])
            nc.gpsimd.dma_start(xb_s[:, CI:], xb_d[:, CI:])
            nc.gpsimd.dma_start(sel2_s[:], sel2_d)

            weff = w1_s  # iteration 0: coef folded into a 0.1 scale of s

            for r in range(NROUTE):
                last = r == NROUTE - 1
                sfx = "_l" if last else ""

                # ---- s partial: [b,(u,o)] += xp^T @ weff over (c,i) tiles ----
                s_stage = wp.tile([128, 2 * UO], F16, tag="s_stage" + sfx)
                s_ps = sps.tile([128, 2 * 256], F32, tag="s_ps")
                scale0 = 0.1 if r == 0 else 1.0
                for g in range(2):
                    for k in range(K):
                        nc.tensor.matmul(
                            s_ps[:, g * 256: g * 256 + UO],
                            lhsT=xp_s[:, k * B + g * 128: k * B + (g + 1) * 128],
                            rhs=weff[:, k * UO:(k + 1) * UO],
                            start=(k == 0), stop=(k == K - 1),
                        )
                    if g == 0:
                        if r == 0:
                            nc.vector.tensor_scalar_mul(
                                s_stage[:, :UO], s_ps[:, :UO], scale0
                            )
                        else:
                            nc.vector.tensor_copy(s_stage[:, :UO], s_ps[:, :UO])
                    else:
                        nc.scalar.activation(
                            s_stage[:, UO:],
                            s_ps[:, 256: 256 + UO],
                            mybir.ActivationFunctionType.Copy,
                            scale=scale0,
                        )

                # ---- collective: AllReduce (inner) / ReduceScatter (last) ----
                cc_in = dram.tile([B, UO], F16, tag="cc_in" + sfx)
                nc.sync.dma_start(
                    cc_in.opt().rearrange("(g p) f -> p g f", g=2), s_stage[:]
                )
                if last:
                    cc_out = dram.tile([BL, UO], F16, tag="cc_out_l")
                    nc.gpsimd.collective_compute(
                        "ReduceScatter",
                        mybir.AluOpType.add,
                        replica_groups=[list(range(N_CORES))],
                        ins=[cc_in.opt()],
                        outs=[cc_out.opt()],
                    )
                    s32 = wp.tile([BL, UO], F16, tag="s32")
                    nc.sync.dma_start(s32[:], cc_out.opt())
                    # squash the 32-row shard and write the sharded output
                    sq32 = wp.tile([BL, UO], F32, tag="sq32")
                    nc.vector.tensor_mul(sq32[:], s32[:], s32[:])
                    n232 = wp.tile([BL, U], F32, tag="n232")
                    nc.vector.reduce_sum(
                        n232[:], sq32[:].rearrange("p (u o) -> p u o", u=U),
                        axis=mybir.AxisListType.X,
                    )
                    rt32 = wp.tile([BL, U], F32, tag="rt32")
                    nc.scalar.sqrt(rt32[:], n232[:])
                    dn32 = wp.tile([BL, U], F32, tag="dn32")
                    nc.vector.tensor_scalar_add(dn32[:], n232[:], 1.0)
                    rd32 = wp.tile([BL, U], F32, tag="rd32")
                    nc.vector.reciprocal(rd32[:], dn32[:])
                    f32t = wp.tile([BL, U], F32, tag="f32t")
                    nc.vector.tensor_mul(f32t[:], rt32[:], rd32[:])
                    vlast = wp.tile([BL, UO], F32, tag="vlast")
                    nc.vector.tensor_mul(
                        vlast[:].rearrange("p (u o) -> p u o", u=U),
                        s32[:].rearrange("p (u o) -> p u o", u=U),
                        f32t[:].unsqueeze(2).broadcast_to((BL, U, O)),
                    )
                    nc.sync.dma_start(
                        out_d.rearrange("p u o one -> p (u o one)"), vlast[:]
                    )
                    break

                cc_out = dram.tile([B, UO], F16, tag="cc_out", addr_space="Shared")
                nc.gpsimd.collective_compute(
                    "AllReduce",
                    mybir.AluOpType.add,
                    replica_groups=[list(range(N_CORES))],
                    ins=[cc_in.opt()],
                    outs=[cc_out.opt()],
                )
                s_sb = wp.tile([128, 2 * UO], F16, tag="s_sb")
                nc.sync.dma_start(
                    s_sb[:].rearrange("p (g f) -> p g f", g=2),
                    cc_out.opt().rearrange("(g p) f -> p g f", g=2),
                )

                # ---- squash: v = s * sqrt(n2) / (1 + n2), split across engines ----
                sq = wp.tile([128, 2 * UO], F32, tag="sq")
                n2 = wp.tile([128, 2 * U], F32, tag="n2")
                sq_engs = [nc.vector, nc.gpsimd]
                for t in range(2):
                    sq_engs[t].tensor_mul(
                        sq[:, t * UO:(t + 1) * UO],
                        s_sb[:, t * UO:(t + 1) * UO],
                        s_sb[:, t * UO:(t + 1) * UO],
                    )
                nc.vector.reduce_sum(
                    n2[:], sq[:].rearrange("p (t u o) -> p (t u) o", t=2, u=U),
                    axis=mybir.AxisListType.X,
                )
                rt = wp.tile([128, 2 * U], F32, tag="rt")
                nc.scalar.sqrt(rt[:], n2[:])
                # prewarm the Exp ACT table while G runs (dep on rt orders it)
                nc.scalar.activation(
                    scr[:, 1:2], rt[:, 0:1], mybir.ActivationFunctionType.Exp
                )
                dn = wp.tile([128, 2 * U], F32, tag="dn")
                nc.vector.tensor_scalar_add(dn[:], n2[:], 1.0)
                rd = wp.tile([128, 2 * U], F32, tag="rd")
                nc.vector.reciprocal(rd[:], dn[:])
                f = wp.tile([128, 2 * U], F32, tag="f")
                nc.vector.tensor_mul(f[:], rt[:], rd[:])
                for t in range(2):
                    sq_engs[t].tensor_mul(
                        v_s[:, t * UO:(t + 1) * UO].rearrange("p (u o) -> p u o", u=U),
                        s_sb[:, t * UO:(t + 1) * UO].rearrange("p (u o) -> p u o", u=U),
                        f[:, t * U:(t + 1) * U].unsqueeze(2).broadcast_to((128, U, O)),
                    )

                # ---- G[(c,i),(u,o)] = sum_b x*v : t-outer so t=0 starts early ----
                g_ps = [
                    gps.tile([128, 3 * 256], F32, tag=f"g_ps{j}", name=f"g_ps{j}")
                    for j in range(3)
                ]
                for m in range(K):
                    j, mm = divmod(m, 3)
                    for t in range(2):
                        nc.tensor.matmul(
                            g_ps[j][:, mm * 256: mm * 256 + UO],
                            lhsT=xb_s[:, t * CI + m * 128: t * CI + (m + 1) * 128],
                            rhs=v_s[:, t * UO:(t + 1) * UO],
                            start=(t == 0), stop=(t == 1),
                        )
                # ---- pm = G*W ; pm2 = sum_o pm  (per j-block, engine split) ----
                pm = wp.tile([128, K * UO], F32, tag="pm")
                for j in range(3):
                    nc.vector.tensor_mul(
                        pm[:, 3 * j * UO: 3 * (j + 1) * UO].rearrange(
                            "p (m q) -> p m q", m=3
                        ),
                        g_ps[j][:].rearrange("p (m q) -> p m q", m=3)[:, :, :UO],
                        w1_s[:, 3 * j * UO: 3 * (j + 1) * UO].rearrange(
                            "p (m q) -> p m q", m=3
                        ),
                    )
                    nc.vector.reduce_sum(
                        pm2_s[:, 3 * j * U: 3 * (j + 1) * U].rearrange(
                            "p (m u) -> p m u", m=3
                        ),
                        pm[:, 3 * j * UO: 3 * (j + 1) * UO].rearrange(
                            "p (m u o) -> p m u o", m=3, u=U
                        ),
                        axis=mybir.AxisListType.X,
                    )
                # ---- agr replicated to all 128 partitions via sel2 matmul ----
                a_ps = aps.tile([128, K * U], F32, tag="a_ps")
                nc.tensor.matmul(
                    a_ps[:], lhsT=sel2_s[:], rhs=pm2_s[:], start=True, stop=True
                )

                # ---- logits update + softmax over u (c-local) ----
                if r == 0:
                    nc.vector.tensor_copy(b_state[:], a_ps[:])
                else:
                    nc.vector.tensor_add(b_state[:], b_state[:], a_ps[:])
                eb = wp.tile([128, K * U], F32, tag="eb")
                nc.scalar.activation(eb[:], b_state[:], mybir.ActivationFunctionType.Exp)
                # prewarm the Sqrt ACT table for the next squash
                nc.scalar.activation(
                    scr[:, 0:1], eb[:, 0:1], mybir.ActivationFunctionType.Sqrt
                )
                den = wp.tile([128, K], F32, tag="den")
                nc.vector.reduce_sum(
                    den[:], eb[:].rearrange("p (k u) -> p k u", k=K),
                    axis=mybir.AxisListType.X,
                )
                rden = wp.tile([128, K], F32, tag="rden")
                nc.vector.reciprocal(rden[:], den[:])
                cnorm = wp.tile([128, K * U], F32, tag="cnorm")
                nc.vector.tensor_mul(
                    cnorm[:].rearrange("p (k u) -> p k u", k=K),
                    eb[:].rearrange("p (k u) -> p k u", k=K),
                    rden[:].unsqueeze(2).broadcast_to((128, K, U)),
                )

                # ---- W_eff = W * coef, split vector/gpsimd ----
                ksplit = 5
                for eng, k0, k1 in ((nc.vector, 0, ksplit), (nc.gpsimd, ksplit, K)):
                    eng.tensor_mul(
                        weff_s[:, k0 * UO: k1 * UO].rearrange(
                            "p (k u o) -> p k u o", k=k1 - k0, u=U
                        ),
                        w1_s[:, k0 * UO: k1 * UO].rearrange(
                            "p (k u o) -> p k u o", k=k1 - k0, u=U
                        ),
                        cnorm[:, k0 * U: k1 * U].rearrange("p (k u) -> p k u", k=k1 - k0)
                        .unsqueeze(3).broadcast_to((128, k1 - k0, U, O)),
                    )
                weff = weff_s

    nc.compile()
    return nc


_PROGRAM_CACHE = {}


def _get_program():
    if "nc" not in _PROGRAM_CACHE:
        _PROGRAM_CACHE["nc"] = _build_program()
    return _PROGRAM_CACHE["nc"]


def _make_in_maps(x, W):
    x = np.ascontiguousarray(x, dtype=np.float32)
    W = np.ascontiguousarray(W, dtype=np.float32)
    bf16 = ml_dtypes.bfloat16
    sel2 = np.zeros((128, 128), dtype=np.float32)
    for p in range(128):
        g = p // IU
        sel2[p, g * IU:(g + 1) * IU] = 1.0 / B

    in_maps = []
    for core in range(N_CORES):
        c0 = core * CL
        xc = x[:, :, c0:c0 + CL]                    # [B, I, CL]
        Wc = W[c0:c0 + CL]                          # [CL, U, O, I]
        # xp[p, k*B + b] = x[b, i, c], ci = k*128+p = c_rel*8+i
        xp = xc.transpose(2, 1, 0).reshape(CI, B)
        xp = np.ascontiguousarray(
            xp.reshape(K, 128, B).transpose(1, 0, 2).reshape(128, K * B)
        ).astype(bf16)
        # xb[p, t*CI + ci] = x[t*128+p, i, c]
        xb = xc.transpose(0, 2, 1).reshape(B, CI)
        xb = np.ascontiguousarray(
            xb.reshape(2, 128, CI).transpose(1, 0, 2).reshape(128, 2 * CI)
        ).astype(bf16)
        # w1[p, k*UO + uo] = W[c, u, o, i]
        w1 = Wc.transpose(0, 3, 1, 2).reshape(CI, UO)
        w1 = np.ascontiguousarray(
            w1.reshape(K, 128, UO).transpose(1, 0, 2).reshape(128, K * UO)
        ).astype(bf16)
        in_maps.append({"xp": xp, "xb": xb, "w1": w1, "sel2": sel2})
    return in_maps


def kernel(x, W, _trace=False, _trace_kwargs=None):
    nc = _get_program()
    in_maps = _make_in_maps(x, W)
    res = run_bass_kernel_spmd(
        nc, in_maps, core_ids=list(range(N_CORES)), trace=_trace,
        **(_trace_kwargs or {}),
    )
    out = np.concatenate(
        [res.results[q]["out"].astype(np.float32) for q in range(N_CORES)], axis=0
    ).reshape(B, U, O, 1)
    if _trace:
        kernel.last_results = res
    return out


# revision 10
# speedup vs baseline: 1.3410x; 1.1169x over previous
"""DigitCaps dynamic-routing kernel for 8 TRN2 NeuronCores.

Strategy: shard the C=1152 input capsules across the 8 cores (144 each) and
keep the full batch B=256 on every core.  The routing iterations use the
factored form (never materializing u_hat = x @ W, which would be 189 MB):

  s[b,u,o]    = sum_{c,i} x[b,i,c] * (coef[c,u] * W[c,u,o,i])     (matmul, K=(c,i))
  v           = squash(s)
  G[ci,uo]    = sum_b x[b,i,c] * v[b,u,o]                          (matmul, K=b)
  agr[ci',u]  = (1/B) * sum_{o} W*G summed over i via sel2 matmul  (replicated
                to all 128 (c,i) partitions so softmax runs there directly)
  b_logits   += agr ; coef = softmax_u(b_logits)                   (c-local)

Cross-core traffic per routing iteration: one fp16 AllReduce of the partial
s ([256,160], 80KB) — the collective does the 8-way sum, so no gather DMAs or
local tree-sum.  The final iteration uses a ReduceScatter instead: each core
squashes only its 32-row batch shard and returns a sharded output that the
host concatenates.

All matmul operands are bf16 (x, W, W_eff, v); accumulation stays fp32 in
PSUM.  Measured output error ~2.8e-3 vs the fp32 reference (tolerance 2e-2).
Iteration 0's uniform coef=0.1 is applied as a 0.1 scale in the PSUM->wire
copy of s.  Elementwise work is split across the vector and gpsimd engines;
the G accumulators use 3 PSUM tiles so the agreement multiply overlaps the
tail of the G matmuls.
"""

import os
import sys

# Prefer the Mesh collective algorithm (RDH measured slower at this size).
os.environ.setdefault("NEURON_RT_DBG_RDH_CC", "0")

if "/opt/trn_rl_repo" not in sys.path:
    sys.path.insert(0, "/opt/trn_rl_repo")

import numpy as np
import ml_dtypes

import concourse.bacc as bacc
import concourse.tile as tile
from concourse import mybir
from concourse.bass_utils import run_bass_kernel_spmd

F32 = mybir.dt.float32
F16 = mybir.dt.float16
BF16 = mybir.dt.bfloat16

B = 256          # batch
IU = 8           # in_unit (i)
C = 1152         # input capsules
U = 10           # output capsules
O = 16           # unit size
N_CORES = 8
CL = C // N_CORES          # 144 local capsules
CI = CL * IU               # 1152 local (c,i) rows
K = CI // 128              # 9 contraction tiles
UO = U * O                 # 160
BL = B // N_CORES          # 32 batch rows per core in the scattered output
NROUTE = 4


def _build_program():
    nc = bacc.Bacc(
        "TRN2",
        target_bir_lowering=False,
        debug=False,
        enable_asserts=False,
        num_devices=N_CORES,
    )

    xp_d = nc.dram_tensor("xp", [128, K * B], BF16, kind="ExternalInput").ap()
    xb_d = nc.dram_tensor("xb", [128, 2 * CI], BF16, kind="ExternalInput").ap()
    w1_d = nc.dram_tensor("w1", [128, K * UO], BF16, kind="ExternalInput").ap()
    sel2_d = nc.dram_tensor("sel2", [128, 128], F32, kind="ExternalInput").ap()
    out_d = nc.dram_tensor("out", [BL, U, O, 1], F32, kind="ExternalOutput").ap()

    with tile.TileContext(nc) as tc:
        with (
            tc.tile_pool(name="persist", bufs=1) as pp,
            tc.tile_pool(name="work", bufs=2) as wp,
            tc.tile_pool(name="sps", bufs=1, space="PSUM") as sps,
            tc.tile_pool(name="gps", bufs=1, space="PSUM") as gps,
            tc.tile_pool(name="aps", bufs=1, space="PSUM") as aps,
            tc.tile_pool(name="dram", bufs=2, space="DRAM") as dram,
        ):
            # ---- persistent tiles ----
            xp_s = pp.tile([128, K * B], BF16, tag="xp")
            xb_s = pp.tile([128, 2 * CI], BF16, tag="xb")
            w1_s = pp.tile([128, K * UO], BF16, tag="w1")
            weff_s = pp.tile([128, K * UO], BF16, tag="weff")
            v_s = pp.tile([128, 2 * UO], BF16, tag="v")
            sel2_s = pp.tile([128, 128], F32, tag="sel2")
            pm2_s = pp.tile([128, K * U], F32, tag="pm2")
            b_state = pp.tile([128, K * U], F32, tag="bstate")
            scr = pp.tile([128, 2], F32, tag="scr")   # ACT table prewarm scratch

            nc.gpsimd.memset(scr[:], 1.0)

            # ---- input loads: xp/w1 first (iter-0 s-matmul), xb/sel2 later ----
            kb3, ku3 = 3 * B, 3 * UO
            for j in range(3):
                nc.sync.dma_start(
                    xp_s[:, j * kb3:(j + 1) * kb3], xp_d[:, j * kb3:(j + 1) * kb3]
                )
                nc.scalar.dma_start(
                    w1_s[:, j * ku3:(j + 1) * ku3], w1_d[:, j * ku3:(j + 1) * ku3]
                )
            weff = w1_s  # iteration 0: coef folded into a 0.1 scale of s

            for r in range(NROUTE):
                last = r == NROUTE - 1
                sfx = "_l" if last else ""

                # ---- s partial: [b,(u,o)] += xp^T @ weff over (c,i) tiles ----
                s_stage = wp.tile([128, 2 * UO], F16, tag="s_stage" + sfx)
                s_ps = sps.tile([128, 2 * 256], F32, tag="s_ps")
                scale0 = 0.1 if r == 0 else 1.0
                for g in range(2):
                    for k in range(K):
                        nc.tensor.matmul(
                            s_ps[:, g * 256: g * 256 + UO],
                            lhsT=xp_s[:, k * B + g * 128: k * B + (g + 1) * 128],
                            rhs=weff[:, k * UO:(k + 1) * UO],
                            start=(k == 0), stop=(k == K - 1),
                        )
                    if g == 0:
                        if r == 0:
                            nc.vector.tensor_scalar_mul(
                                s_stage[:, :UO], s_ps[:, :UO], scale0
                            )
                        else:
                            nc.vector.tensor_copy(s_stage[:, :UO], s_ps[:, :UO])
                    else:
                        nc.scalar.activation(
                            s_stage[:, UO:],
                            s_ps[:, 256: 256 + UO],
                            mybir.ActivationFunctionType.Copy,
                            scale=scale0,
                        )

                # ---- collective: AllReduce (inner) / ReduceScatter (last) ----
                cc_in = dram.tile([B, UO], F16, tag="cc_in" + sfx)
                cin_view = cc_in.opt().rearrange("(g p) f -> p g f", g=2)
                nc.sync.dma_start(cin_view[:, 0], s_stage[:, :UO])
                nc.scalar.dma_start(cin_view[:, 1], s_stage[:, UO:])
                if last:
                    cc_out = dram.tile([BL, UO], F16, tag="cc_out_l")
                    nc.gpsimd.collective_compute(
                        "ReduceScatter",
                        mybir.AluOpType.add,
                        replica_groups=[list(range(N_CORES))],
                        ins=[cc_in.opt()],
                        outs=[cc_out.opt()],
                    )
                    s32 = wp.tile([BL, UO], F16, tag="s32")
                    nc.sync.dma_start(s32[:], cc_out.opt())
                    # squash the 32-row shard and write the sharded output
                    sq32 = wp.tile([BL, UO], F32, tag="sq32")
                    nc.vector.tensor_mul(sq32[:], s32[:], s32[:])
                    n232 = wp.tile([BL, U], F32, tag="n232")
                    nc.vector.reduce_sum(
                        n232[:], sq32[:].rearrange("p (u o) -> p u o", u=U),
                        axis=mybir.AxisListType.X,
                    )
                    rt32 = wp.tile([BL, U], F32, tag="rt32")
                    nc.scalar.sqrt(rt32[:], n232[:])
                    dn32 = wp.tile([BL, U], F32, tag="dn32")
                    nc.vector.tensor_scalar_add(dn32[:], n232[:], 1.0)
                    rd32 = wp.tile([BL, U], F32, tag="rd32")
                    nc.vector.reciprocal(rd32[:], dn32[:])
                    f32t = wp.tile([BL, U], F32, tag="f32t")
                    nc.vector.tensor_mul(f32t[:], rt32[:], rd32[:])
                    vlast = wp.tile([BL, UO], F32, tag="vlast")
                    nc.vector.tensor_mul(
                        vlast[:].rearrange("p (u o) -> p u o", u=U),
                        s32[:].rearrange("p (u o) -> p u o", u=U),
                        f32t[:].unsqueeze(2).broadcast_to((BL, U, O)),
                    )
                    nc.sync.dma_start(
                        out_d.rearrange("p u o one -> p (u o one)"), vlast[:]
                    )
                    break

                cc_out = dram.tile([B, UO], F16, tag="cc_out", addr_space="Shared")
                nc.gpsimd.collective_compute(
                    "AllReduce",
                    mybir.AluOpType.add,
                    replica_groups=[list(range(N_CORES))],
                    ins=[cc_in.opt()],
                    outs=[cc_out.opt()],
                )
                if r == 0:
                    # xb/sel2 aren't needed until after this collective; loading
                    # them now keeps the HBM queues free for the xp/w1 prefix
                    nc.gpsimd.dma_start(xb_s[:, :CI], xb_d[:, :CI])
                    nc.gpsimd.dma_start(xb_s[:, CI:], xb_d[:, CI:])
                    nc.gpsimd.dma_start(sel2_s[:], sel2_d)
                s_sb = wp.tile([128, 2 * UO], F16, tag="s_sb")
                cc_view = cc_out.opt().rearrange("(g p) f -> p g f", g=2)
                nc.sync.dma_start(s_sb[:, :UO], cc_view[:, 0])
                nc.scalar.dma_start(s_sb[:, UO:], cc_view[:, 1])

                # ---- squash: v = s * sqrt(n2) / (1 + n2), split across engines ----
                sq = wp.tile([128, 2 * UO], F32, tag="sq")
                n2 = wp.tile([128, 2 * U], F32, tag="n2")
                sq_engs = [nc.vector, nc.gpsimd]
                for t in range(2):
                    sq_engs[t].tensor_mul(
                        sq[:, t * UO:(t + 1) * UO],
                        s_sb[:, t * UO:(t + 1) * UO],
                        s_sb[:, t * UO:(t + 1) * UO],
                    )
                nc.vector.reduce_sum(
                    n2[:], sq[:].rearrange("p (t u o) -> p (t u) o", t=2, u=U),
                    axis=mybir.AxisListType.X,
                )
                rt = wp.tile([128, 2 * U], F32, tag="rt")
                nc.scalar.sqrt(rt[:], n2[:])
                # prewarm the Exp ACT table while G runs (dep on rt orders it)
                nc.scalar.activation(
                    scr[:, 1:2], rt[:, 0:1], mybir.ActivationFunctionType.Exp
                )
                dn = wp.tile([128, 2 * U], F32, tag="dn")
                nc.vector.tensor_scalar_add(dn[:], n2[:], 1.0)
                rd = wp.tile([128, 2 * U], F32, tag="rd")
                nc.vector.reciprocal(rd[:], dn[:])
                f = wp.tile([128, 2 * U], F32, tag="f")
                nc.vector.tensor_mul(f[:], rt[:], rd[:])
                for t in range(2):
                    sq_engs[t].tensor_mul(
                        v_s[:, t * UO:(t + 1) * UO].rearrange("p (u o) -> p u o", u=U),
                        s_sb[:, t * UO:(t + 1) * UO].rearrange("p (u o) -> p u o", u=U),
                        f[:, t * U:(t + 1) * U].unsqueeze(2).broadcast_to((128, U, O)),
                    )

                # ---- G[(c,i),(u,o)] = sum_b x*v : t-outer so t=0 starts early ----
                g_ps = [
                    gps.tile([128, 3 * 256], F32, tag=f"g_ps{j}", name=f"g_ps{j}")
                    for j in range(3)
                ]
                for m in range(K):
                    j, mm = divmod(m, 3)
                    for t in range(2):
                        nc.tensor.matmul(
                            g_ps[j][:, mm * 256: mm * 256 + UO],
                            lhsT=xb_s[:, t * CI + m * 128: t * CI + (m + 1) * 128],
                            rhs=v_s[:, t * UO:(t + 1) * UO],
                            start=(t == 0), stop=(t == 1),
                        )
                # ---- pm = G*W ; pm2 = sum_o pm  (per j-block, engine split) ----
                pm = wp.tile([128, K * UO], BF16, tag="pm")
                for j in range(3):
                    nc.vector.tensor_mul(
                        pm[:, 3 * j * UO: 3 * (j + 1) * UO].rearrange(
                            "p (m q) -> p m q", m=3
                        ),
                        g_ps[j][:].rearrange("p (m q) -> p m q", m=3)[:, :, :UO],
                        w1_s[:, 3 * j * UO: 3 * (j + 1) * UO].rearrange(
                            "p (m q) -> p m q", m=3
                        ),
                    )
                    nc.vector.reduce_sum(
                        pm2_s[:, 3 * j * U: 3 * (j + 1) * U].rearrange(
                            "p (m u) -> p m u", m=3
                        ),
                        pm[:, 3 * j * UO: 3 * (j + 1) * UO].rearrange(
                            "p (m u o) -> p m u o", m=3, u=U
                        ),
                        axis=mybir.AxisListType.X,
                    )
                # ---- agr replicated to all 128 partitions via sel2 matmul ----
                a_ps = aps.tile([128, K * U], F32, tag="a_ps")
                nc.tensor.matmul(
                    a_ps[:], lhsT=sel2_s[:], rhs=pm2_s[:], start=True, stop=True
                )

                # ---- logits update + softmax over u (c-local) ----
                if r == 0:
                    nc.vector.tensor_copy(b_state[:], a_ps[:])
                else:
                    nc.vector.tensor_add(b_state[:], b_state[:], a_ps[:])
                eb = wp.tile([128, K * U], F32, tag="eb")
                nc.scalar.activation(eb[:], b_state[:], mybir.ActivationFunctionType.Exp)
                # prewarm the Sqrt ACT table for the next squash
                nc.scalar.activation(
                    scr[:, 0:1], eb[:, 0:1], mybir.ActivationFunctionType.Sqrt
                )
                den = wp.tile([128, K], F32, tag="den")
                nc.vector.reduce_sum(
                    den[:], eb[:].rearrange("p (k u) -> p k u", k=K),
                    axis=mybir.AxisListType.X,
                )
                rden = wp.tile([128, K], F32, tag="rden")
                nc.vector.reciprocal(rden[:], den[:])
                cnorm = wp.tile([128, K * U], F32, tag="cnorm")
                nc.vector.tensor_mul(
                    cnorm[:].rearrange("p (k u) -> p k u", k=K),
                    eb[:].rearrange("p (k u) -> p k u", k=K),
                    rden[:].unsqueeze(2).broadcast_to((128, K, U)),
                )

                # ---- W_eff = W * coef, split vector/gpsimd ----
                ksplit = 6
                for eng, k0, k1 in ((nc.vector, 0, ksplit), (nc.gpsimd, ksplit, K)):
                    eng.tensor_mul(
                        weff_s[:, k0 * UO: k1 * UO].rearrange(
                            "p (k u o) -> p k u o", k=k1 - k0, u=U
                        ),
                        w1_s[:, k0 * UO: k1 * UO].rearrange(
                            "p (k u o) -> p k u o", k=k1 - k0, u=U
                        ),
                        cnorm[:, k0 * U: k1 * U].rearrange("p (k u) -> p k u", k=k1 - k0)
                        .unsqueeze(3).broadcast_to((128, k1 - k0, U, O)),
                    )
                weff = weff_s

    nc.compile()
    return nc


_PROGRAM_CACHE = {}


def _get_program():
    if "nc" not in _PROGRAM_CACHE:
        _PROGRAM_CACHE["nc"] = _build_program()
    return _PROGRAM_CACHE["nc"]


def _make_in_maps(x, W):
    x = np.ascontiguousarray(x, dtype=np.float32)
    W = np.ascontiguousarray(W, dtype=np.float32)
    bf16 = ml_dtypes.bfloat16
    sel2 = np.zeros((128, 128), dtype=np.float32)
    for p in range(128):
        g = p // IU
        sel2[p, g * IU:(g + 1) * IU] = 1.0 / B

    in_maps = []
    for core in range(N_CORES):
        c0 = core * CL
        xc = x[:, :, c0:c0 + CL]                    # [B, I, CL]
        Wc = W[c0:c0 + CL]                          # [CL, U, O, I]
        # xp[p, k*B + b] = x[b, i, c], ci = k*128+p = c_rel*8+i
        xp = xc.transpose(2, 1, 0).reshape(CI, B)
        xp = np.ascontiguousarray(
            xp.reshape(K, 128, B).transpose(1, 0, 2).reshape(128, K * B)
        ).astype(bf16)
        # xb[p, t*CI + ci] = x[t*128+p, i, c]
        xb = xc.transpose(0, 2, 1).reshape(B, CI)
        xb = np.ascontiguousarray(
            xb.reshape(2, 128, CI).transpose(1, 0, 2).reshape(128, 2 * CI)
        ).astype(bf16)
        # w1[p, k*UO + uo] = W[c, u, o, i]
        w1 = Wc.transpose(0, 3, 1, 2).reshape(CI, UO)
        w1 = np.ascontiguousarray(
            w1.reshape(K, 128, UO).transpose(1, 0, 2).reshape(128, K * UO)
        ).astype(bf16)
        in_maps.append({"xp": xp, "xb": xb, "w1": w1, "sel2": sel2})
    return in_maps


def kernel(x, W, _trace=False, _trace_kwargs=None):
    nc = _get_program()
    in_maps = _make_in_maps(x, W)
    res = run_bass_kernel_spmd(
        nc, in_maps, core_ids=list(range(N_CORES)), trace=_trace,
        **(_trace_kwargs or {}),
    )
    out = np.concatenate(
        [res.results[q]["out"].astype(np.float32) for q in range(N_CORES)], axis=0
    ).reshape(B, U, O, 1)
    if _trace:
        kernel.last_results = res
    return out
